# revision 18
# baseline (speedup 1.0000x reference)
"""Multi-head attention (dense_transformer) on 8 TRN2 NeuronCores.

Sharding: data-parallel over batch (2) x tensor-parallel over head groups
(16 heads -> 4 groups of 4). Core (b, g) computes, for batch b:
  Q/K/V for its 4 heads (x @ w_qkv columns), per-head softmax(QK^T/sqrt(d))V,
  and the partial projection  attn_out_g @ w_proj[rows of g]  (+ b_proj/4).
The host sums the 4 partial projections per batch (the "all-reduce after
proj" of the sharding hint, done at gather time) and stacks the 2 batches.

Schedule notes (v4): ScalarE's exp stream (128 x [128,1024] ACTIVATEs,
~1.11us each at full clock) is the wall; everything else hides behind it.
  - Softmax 1/l via DVE copy + reciprocal_approx_fast + tensor_mul; no ACT
    table switches ever (Exp only -> one ACT_TABLE_LOAD total).
  - Keeping the PE duty-cycle moderate matters: a denser PE stream (v3
    experiment, software-pipelined PVs) tripped the chip's P0 power
    downclock and made EVERY engine 1.2x slower. The "idle" PE slices in
    this schedule buy full clock for ACT.
  - Input DMA uses 4 big triggers (one per q-chunk over all 8 contraction
    chunks) because dma_start triggers serialize at ~0.65us each on one
    queue; output DMAs ride the otherwise-idle GpSimd queue, one per
    token-block.
  - Sweep boundaries: S+exp of the next sweep's kb=0 are emitted before the
    norm/Q-weave chain so ACT never waits on it; Q chunks 2/3 ride the
    just-freed acc banks at the boundary; Q1 weaves into sweep 0's stp
    rotation; proj of chunk s weaves one psum-group per iteration into
    sweep s+1.
"""

import numpy as np

DIM = 1024
NUM_HEADS = 16
HEAD_DIM = 64
SCALE = HEAD_DIM ** -0.5
B = 2
N = 2048
NCORES = 8
HPG = 4               # heads per group (tensor-parallel degree 4)
GD = HPG * HEAD_DIM   # 256 dims per head group
CC = DIM // 128       # 8 contraction chunks over the model dim
TB = N // 128         # 16 token blocks
NQ = N // 512         # 4 query chunks
KB = N // 128         # 16 key blocks

_CACHE = {}


def _build_nc():
    from contextlib import ExitStack

    import concourse.tile as tile
    from concourse import bacc, mybir

    f32 = mybir.dt.float32
    bf16 = mybir.dt.bfloat16
    EXP = mybir.ActivationFunctionType.Exp

    nc = bacc.Bacc("TRN2", target_bir_lowering=False, debug=False,
                   enable_asserts=False)

    # Matmul operands arrive pre-rounded to bf16 on the host.
    xt = nc.dram_tensor("xt", [DIM, N], bf16, kind="ExternalInput").ap()
    wq = nc.dram_tensor("wq", [DIM, GD], bf16, kind="ExternalInput").ap()
    wk = nc.dram_tensor("wk", [DIM, GD], bf16, kind="ExternalInput").ap()
    wv = nc.dram_tensor("wv", [DIM, GD], bf16, kind="ExternalInput").ap()
    wp = nc.dram_tensor("wp", [GD, DIM], bf16, kind="ExternalInput").ap()
    out = nc.dram_tensor("out", [N, DIM], bf16, kind="ExternalOutput").ap()

    with tile.TileContext(nc) as tc, ExitStack() as ctx:
        big = ctx.enter_context(tc.tile_pool(name="big", bufs=1))
        pts = ctx.enter_context(tc.tile_pool(name="pts", bufs=2))
        outst = ctx.enter_context(tc.tile_pool(name="outst", bufs=2))
        small = ctx.enter_context(tc.tile_pool(name="small", bufs=2))
        # All of PSUM: stp0/stp1 are the two [128,1024] S^T/exp slots (2 banks
        # each, also cycled by the K/Q/proj matmul groups), acc0-3 hold the
        # per-head PV accumulators (1 bank each; V scratch in the prefix).
        psum = ctx.enter_context(tc.tile_pool(name="psum", bufs=1, space="PSUM"))

        # ---- resident inputs ------------------------------------------------
        # wv first (V runs first), xt in one trigger per q-chunk (all 8
        # contraction chunks at once: trigger issue is the bottleneck, not
        # bandwidth), wk/wq between the first xt chunks, wp last.
        wv_sb = big.tile([128, CC, GD], bf16, tag="wv")
        nc.sync.dma_start(wv_sb[:], wv.rearrange("(cc p) d -> p cc d", p=128))
        xt_r = xt.rearrange("(cc p) n -> p cc n", p=128)
        xt_sb = big.tile([128, CC, N], bf16, tag="xt")
        # token-block granularity for the first q-chunk so V(0) starts ~4us
        # earlier; whole-chunk triggers after that.
        for t in range(4):
            nc.sync.dma_start(xt_sb[:, :, t * 128:(t + 1) * 128],
                              xt_r[:, :, t * 128:(t + 1) * 128])
        wk_sb = big.tile([128, CC, GD], bf16, tag="wk")
        nc.sync.dma_start(wk_sb[:], wk.rearrange("(cc p) d -> p cc d", p=128))
        wq_sb = big.tile([128, CC, GD], bf16, tag="wq")
        nc.sync.dma_start(wq_sb[:], wq.rearrange("(cc p) d -> p cc d", p=128))
        for j in range(1, NQ):
            nc.sync.dma_start(xt_sb[:, :, j * 512:(j + 1) * 512],
                              xt_r[:, :, j * 512:(j + 1) * 512])
        wp_sb = big.tile([128, 2, DIM], bf16, tag="wp")
        nc.sync.dma_start(wp_sb[:], wp.rearrange("(dc p) d -> p dc d", p=128))

        qt_sb = big.tile([128, 2, N], bf16, tag="qt")   # Q^T: [d, tok]
        kt_sb = big.tile([128, 2, N], bf16, tag="kt")   # K^T: [d, tok]
        # V stored per (token-block, head) as [V_h | ones] (128 cols): the PV
        # matmul uses the whole 128-col block as lhsT (M=128) so PSUM rows
        # 0-63 get O^T_h and rows 64-127 get the softmax denom replicated 64x.
        v_sb = big.tile([128, TB, HPG, 128], bf16, tag="v")
        nc.vector.memset(v_sb[:, :, :, 64:128], 1.0)
        ot_sb = big.tile([128, 2, N], bf16, tag="ot")   # attn-out^T: [d, tok]

        # ---- prefix: V (all), K (all), Q chunk 0 ----------------------------
        # V[tok,d]: lhsT = x^T chunk [c,tok], rhs = wv chunk [c,d]; the acc
        # banks are free scratch until the first PV.
        for tb in range(TB):
            ps = psum.tile([128, 512], f32, tag=f"acc{tb % 4}", name=f"vps{tb}")
            for cc in range(CC):
                nc.tensor.matmul(
                    ps[:, 0:GD],
                    xt_sb[:, cc, tb * 128:(tb + 1) * 128],
                    wv_sb[:, cc, :],
                    start=(cc == 0), stop=(cc == CC - 1),
                )
            nc.vector.tensor_copy(
                v_sb[:, tb, :, 0:64],
                ps[:, 0:GD].rearrange("p (h d) -> p h d", h=HPG))

        def qk_group(w_sb, dst, mb, nq, tag):
            # dst^T[d,tok] for 128 d-dims x 512 tokens: lhsT = w chunk [c,d],
            # rhs = x^T chunk [c,tok].
            ps = psum.tile([128, 512], f32, tag=tag, name=f"qk_{tag}_{mb}_{nq}")
            for cc in range(CC):
                nc.tensor.matmul(
                    ps[:],
                    w_sb[:, cc, mb * 128:(mb + 1) * 128],
                    xt_sb[:, cc, nq * 512:(nq + 1) * 512],
                    start=(cc == 0), stop=(cc == CC - 1),
                )
            nc.vector.tensor_copy(dst[:, mb, nq * 512:(nq + 1) * 512], ps[:])

        for j in range(NQ):                      # K, all chunks
            for mb in range(2):
                qk_group(wk_sb, kt_sb, mb, j, f"stp{(2 * j + mb) % 2}")
        for mb in range(2):                      # Q chunk 0
            qk_group(wq_sb, qt_sb, mb, 0, f"stp{mb}")

        # ---- attention sweeps with woven projection -------------------------
        def s_exp(nq, kb):
            # Per head pair: S^T block via 2 row-tiled concurrent matmuls
            # (lhsT = K^T_h [d=64, k-block], rhs = Q^T_h), then one
            # P^T = exp(SCALE*S^T) over the pair's [128,1024] PSUM slot.
            qsl = slice(nq * 512, (nq + 1) * 512)
            pts_kb = []
            for pr in range(2):
                stp = psum.tile([128, 1024], f32, tag=f"stp{pr}",
                                name=f"stp{pr}_{nq}_{kb}")
                for hh in range(2):
                    h = 2 * pr + hh
                    po = 64 * (h % 2)
                    nc.tensor.matmul(
                        stp[:, hh * 512:(hh + 1) * 512],
                        kt_sb[po:po + 64, h // 2, kb * 128:(kb + 1) * 128],
                        qt_sb[po:po + 64, h // 2, qsl],
                        start=True, stop=True,
                    )
                pt = pts.tile([128, 1024], bf16, tag=f"pt{pr}",
                              name=f"pt{pr}_{nq}_{kb}")
                nc.scalar.activation(pt[:], stp[:], EXP, scale=SCALE)
                pts_kb.append(pt)
            return pts_kb

        def pv(accs, pts_kb, kb):
            for h in range(HPG):
                nc.tensor.matmul(
                    accs[h][:], v_sb[:, kb, h, :],
                    pts_kb[h // 2][:, (h % 2) * 512:(h % 2 + 1) * 512],
                    start=(kb == 0), stop=(kb == KB - 1),
                )

        def proj_mms(tb, nb, tag):
            ps = psum.tile([128, 512], f32, tag=tag, name=f"pjps{tb}_{nb}")
            for dc in range(2):
                nc.tensor.matmul(
                    ps[:],
                    ot_sb[:, dc, tb * 128:(tb + 1) * 128],
                    wp_sb[:, dc, nb * 512:(nb + 1) * 512],
                    start=(dc == 0), stop=(dc == 1),
                )
            return ps

        def norm_h(nq, accs, h, tail=False):
            qsl = slice(nq * 512, (nq + 1) * 512)
            po = 64 * (h % 2)
            den = small.tile([64, 512], f32, tag="den", name=f"den{h}_{nq}")
            if tail:
                nc.scalar.copy(den[:], accs[h][64:128, :])
            else:
                nc.vector.tensor_copy(den[:], accs[h][64:128, :])
            rec = small.tile([64, 512], f32, tag="rec", name=f"rec{h}_{nq}")
            nc.vector.reciprocal_approx_fast(rec[:], den[:])
            nc.vector.tensor_mul(
                ot_sb[po:po + 64, h // 2, qsl], accs[h][0:64, :], rec[:])

        def norm(nq, accs, tail=False):
            # ot = O^T * (1/l). The custom-DVE fast reciprocal (~51 ULP) is
            # broken for PSUM-src at base_partition 64 and for partition-
            # shifted outputs, so stage the denominator into SBUF with a plain
            # copy first (on ACT at the tail where it is idle, on DVE at the
            # boundaries where ACT is the bottleneck).
            for h in range(HPG):
                norm_h(nq, accs, h, tail)

        previews = []           # S/exp of the next sweep's first kbs, pre-issued
        for nq in range(NQ):
            accs = [psum.tile([128, 512], f32, tag=f"acc{h}", name=f"acc{h}_{nq}")
                    for h in range(HPG)]
            ob = None
            for kb in range(KB):
                pts_kb = previews.pop(0) if previews else s_exp(nq, kb)
                pv(accs, pts_kb, kb)
                # woven work (runs in the PE slack while ACT chews the exps):
                if nq == 0 and kb in (6, 9):
                    # Q chunk 1 into the stp rotation (pays its duration once).
                    qk_group(wq_sb, qt_sb, (0 if kb == 6 else 1), 1,
                             f"stp{0 if kb == 6 else 1}")
                if nq >= 1 and 2 <= kb <= 9:
                    # projection of the previous q-chunk: one psum-group per
                    # iteration, one output DMA per token-block on the idle
                    # GpSimd queue. (Keeping the slot alternation stp0/stp1 —
                    # pinning to stp1 removes the stalls but the denser PE
                    # stream then trips the P0 downclock: net 1.2x loss.)
                    tb = 4 * (nq - 1) + (kb - 2) // 2
                    nb = (kb - 2) % 2
                    if nb == 0:
                        ob = outst.tile([128, 1024], bf16, tag="ob",
                                        name=f"ob{tb}")
                    ps = proj_mms(tb, nb, f"stp{nb}")
                    nc.vector.tensor_copy(
                        ob[:, nb * 512:(nb + 1) * 512], ps[:])
                    if nb == 1:
                        nc.gpsimd.dma_start(
                            out[tb * 128:(tb + 1) * 128, :], ob[:])
            # ---- sweep boundary ---------------------------------------------
            # Pre-issue the next sweep's first two kbs of S/exp (~6 exps of
            # ACT buffer) so the exp stream never waits behind the norm ->
            # Q-weave -> PE chain below.
            if nq + 1 < NQ:
                previews = [s_exp(nq + 1, 0)]
            if nq < NQ - 1:
                norm(nq, accs)   # nq==3's norm happens dc-split in the tail
            # Q chunks 2/3 ride the boundary in the just-freed acc banks (the
            # next sweep's PVs have PE slack to absorb the delay).
            if nq + 2 <= NQ - 1:
                for mb in range(2):
                    qk_group(wq_sb, qt_sb, mb, nq + 2, f"acc{mb}")
        # ---- tail: norm of q-chunk 3, then its projection -------------------
        norm(NQ - 1, accs, tail=True)
        for i, tb in enumerate(range(4 * (NQ - 1), 4 * NQ)):
            ob = outst.tile([128, 1024], bf16, tag="ob", name=f"ob{tb}")
            for nb in range(2):
                ps = proj_mms(tb, nb, f"acc{(2 * i + nb) % 4}")
                nc.vector.tensor_copy(ob[:, nb * 512:(nb + 1) * 512], ps[:])
            eng = nc.gpsimd if i % 2 == 0 else nc.sync
            eng.dma_start(out[tb * 128:(tb + 1) * 128, :], ob[:])

    nc.compile()
    return nc


def get_nc():
    if "nc" not in _CACHE:
        _CACHE["nc"] = _build_nc()
    return _CACHE["nc"]


def to_bf16(a):
    import ml_dtypes
    return np.ascontiguousarray(np.asarray(a, dtype=np.float32)).astype(ml_dtypes.bfloat16)


def make_in_maps(x, w_qkv, w_proj, b_proj):
    x = np.ascontiguousarray(np.asarray(x, dtype=np.float32))
    w_qkv = np.asarray(w_qkv, dtype=np.float32)
    w_proj = np.asarray(w_proj, dtype=np.float32)
    b_proj = np.asarray(b_proj, dtype=np.float32)

    wr = w_qkv.reshape(DIM, 3, NUM_HEADS, HEAD_DIM)
    xts = [to_bf16(x[b].T) for b in range(B)]

    in_maps = []
    for core in range(NCORES):
        b, g = divmod(core, HPG)
        h0, h1 = HPG * g, HPG * (g + 1)
        in_maps.append({
            "xt": xts[b],
            "wq": to_bf16(wr[:, 0, h0:h1, :].reshape(DIM, GD)),
            "wk": to_bf16(wr[:, 1, h0:h1, :].reshape(DIM, GD)),
            "wv": to_bf16(wr[:, 2, h0:h1, :].reshape(DIM, GD)),
            "wp": to_bf16(w_proj[g * GD:(g + 1) * GD, :]),
        })
    return in_maps


def gather_out(results, b_proj):
    b_proj = np.asarray(b_proj, dtype=np.float32)
    parts = [np.asarray(r["out"], dtype=np.float32) for r in results]
    return np.stack(
        [sum(parts[b * HPG:(b + 1) * HPG][1:], parts[b * HPG]) + b_proj
         for b in range(B)],
        axis=0,
    ).astype(np.float32)


def kernel(x, w_qkv, w_proj, b_proj):
    from concourse import bass_utils

    nc = get_nc()
    in_maps = make_in_maps(x, w_qkv, w_proj, b_proj)
    res = bass_utils.run_bass_kernel_spmd(nc, in_maps, core_ids=list(range(NCORES)))
    return gather_out(res.results, b_proj)


# revision 19
# speedup vs baseline: 1.0169x; 1.0169x over previous
"""Multi-head attention (dense_transformer) on 8 TRN2 NeuronCores.

Sharding: data-parallel over batch (2) x tensor-parallel over head groups
(16 heads -> 4 groups of 4). Core (b, g) computes, for batch b:
  Q/K/V for its 4 heads (x @ w_qkv columns), per-head softmax(QK^T/sqrt(d))V,
  and the partial projection  attn_out_g @ w_proj[rows of g]  (+ b_proj/4).
The host sums the 4 partial projections per batch (the "all-reduce after
proj" of the sharding hint, done at gather time) and stacks the 2 batches.

Schedule notes (v4): ScalarE's exp stream (128 x [128,1024] ACTIVATEs,
~1.11us each at full clock) is the wall; everything else hides behind it.
  - Softmax 1/l via DVE copy + reciprocal_approx_fast + tensor_mul; no ACT
    table switches ever (Exp only -> one ACT_TABLE_LOAD total).
  - Keeping the PE duty-cycle moderate matters: a denser PE stream (v3
    experiment, software-pipelined PVs) tripped the chip's P0 power
    downclock and made EVERY engine 1.2x slower. The "idle" PE slices in
    this schedule buy full clock for ACT.
  - Input DMA uses 4 big triggers (one per q-chunk over all 8 contraction
    chunks) because dma_start triggers serialize at ~0.65us each on one
    queue; output DMAs ride the otherwise-idle GpSimd queue, one per
    token-block.
  - Sweep boundaries: S+exp of the next sweep's kb=0 are emitted before the
    norm/Q-weave chain so ACT never waits on it; Q chunks 2/3 ride the
    just-freed acc banks at the boundary; Q1 weaves into sweep 0's stp
    rotation; proj of chunk s weaves one psum-group per iteration into
    sweep s+1.
"""

import numpy as np

DIM = 1024
NUM_HEADS = 16
HEAD_DIM = 64
SCALE = HEAD_DIM ** -0.5
B = 2
N = 2048
NCORES = 8
HPG = 4               # heads per group (tensor-parallel degree 4)
GD = HPG * HEAD_DIM   # 256 dims per head group
CC = DIM // 128       # 8 contraction chunks over the model dim
TB = N // 128         # 16 token blocks
NQ = N // 512         # 4 query chunks
KB = N // 128         # 16 key blocks

_CACHE = {}


def _build_nc():
    from contextlib import ExitStack

    import concourse.tile as tile
    from concourse import bacc, mybir

    f32 = mybir.dt.float32
    bf16 = mybir.dt.bfloat16
    EXP = mybir.ActivationFunctionType.Exp

    nc = bacc.Bacc("TRN2", target_bir_lowering=False, debug=False,
                   enable_asserts=False)

    # Matmul operands arrive pre-rounded to bf16 on the host.
    xt = nc.dram_tensor("xt", [DIM, N], bf16, kind="ExternalInput").ap()
    wq = nc.dram_tensor("wq", [DIM, GD], bf16, kind="ExternalInput").ap()
    wk = nc.dram_tensor("wk", [DIM, GD], bf16, kind="ExternalInput").ap()
    wv = nc.dram_tensor("wv", [DIM, GD], bf16, kind="ExternalInput").ap()
    wp = nc.dram_tensor("wp", [GD, DIM], bf16, kind="ExternalInput").ap()
    out = nc.dram_tensor("out", [N, DIM], bf16, kind="ExternalOutput").ap()

    with tile.TileContext(nc) as tc, ExitStack() as ctx:
        big = ctx.enter_context(tc.tile_pool(name="big", bufs=1))
        pts = ctx.enter_context(tc.tile_pool(name="pts", bufs=2))
        outst = ctx.enter_context(tc.tile_pool(name="outst", bufs=2))
        small = ctx.enter_context(tc.tile_pool(name="small", bufs=2))
        # All of PSUM: stp0/stp1 are the two [128,1024] S^T/exp slots (2 banks
        # each, also cycled by the K/Q/proj matmul groups), acc0-3 hold the
        # per-head PV accumulators (1 bank each; V scratch in the prefix).
        psum = ctx.enter_context(tc.tile_pool(name="psum", bufs=1, space="PSUM"))

        # ---- resident inputs ------------------------------------------------
        # wv first (V runs first), xt in one trigger per q-chunk (all 8
        # contraction chunks at once: trigger issue is the bottleneck, not
        # bandwidth), wk/wq between the first xt chunks, wp last.
        # wv in halves and the first chunks of xt at token-block granularity:
        # the V matmul stream gates on these transfers, and the DMA completion
        # semaphore only fires per trigger, so finer slices start V earlier.
        wv_sb = big.tile([128, CC, GD], bf16, tag="wv")
        wv_r = wv.rearrange("(cc p) d -> p cc d", p=128)
        nc.sync.dma_start(wv_sb[:, 0:4, :], wv_r[:, 0:4, :])
        nc.sync.dma_start(wv_sb[:, 4:CC, :], wv_r[:, 4:CC, :])
        xt_r = xt.rearrange("(cc p) n -> p cc n", p=128)
        xt_sb = big.tile([128, CC, N], bf16, tag="xt")
        for t in range(4):
            nc.sync.dma_start(xt_sb[:, :, t * 128:(t + 1) * 128],
                              xt_r[:, :, t * 128:(t + 1) * 128])
        wk_sb = big.tile([128, CC, GD], bf16, tag="wk")
        nc.sync.dma_start(wk_sb[:], wk.rearrange("(cc p) d -> p cc d", p=128))
        wq_sb = big.tile([128, CC, GD], bf16, tag="wq")
        nc.sync.dma_start(wq_sb[:], wq.rearrange("(cc p) d -> p cc d", p=128))
        nc.sync.dma_start(xt_sb[:, :, 512:768], xt_r[:, :, 512:768])
        nc.sync.dma_start(xt_sb[:, :, 768:1024], xt_r[:, :, 768:1024])
        for j in range(2, NQ):
            nc.sync.dma_start(xt_sb[:, :, j * 512:(j + 1) * 512],
                              xt_r[:, :, j * 512:(j + 1) * 512])
        wp_sb = big.tile([128, 2, DIM], bf16, tag="wp")
        nc.sync.dma_start(wp_sb[:], wp.rearrange("(dc p) d -> p dc d", p=128))

        qt_sb = big.tile([128, 2, N], bf16, tag="qt")   # Q^T: [d, tok]
        kt_sb = big.tile([128, 2, N], bf16, tag="kt")   # K^T: [d, tok]
        # V stored per (token-block, head) as [V_h | ones] (128 cols): the PV
        # matmul uses the whole 128-col block as lhsT (M=128) so PSUM rows
        # 0-63 get O^T_h and rows 64-127 get the softmax denom replicated 64x.
        v_sb = big.tile([128, TB, HPG, 128], bf16, tag="v")
        nc.vector.memset(v_sb[:, :, :, 64:128], 1.0)
        ot_sb = big.tile([128, 2, N], bf16, tag="ot")   # attn-out^T: [d, tok]

        # ---- prefix: V (all), K (all), Q chunk 0 ----------------------------
        # V[tok,d]: lhsT = x^T chunk [c,tok], rhs = wv chunk [c,d]; the acc
        # banks are free scratch until the first PV.
        for tb in range(TB):
            ps = psum.tile([128, 512], f32, tag=f"acc{tb % 4}", name=f"vps{tb}")
            for cc in range(CC):
                nc.tensor.matmul(
                    ps[:, 0:GD],
                    xt_sb[:, cc, tb * 128:(tb + 1) * 128],
                    wv_sb[:, cc, :],
                    start=(cc == 0), stop=(cc == CC - 1),
                )
            nc.vector.tensor_copy(
                v_sb[:, tb, :, 0:64],
                ps[:, 0:GD].rearrange("p (h d) -> p h d", h=HPG))

        def qk_group(w_sb, dst, mb, nq, tag):
            # dst^T[d,tok] for 128 d-dims x 512 tokens: lhsT = w chunk [c,d],
            # rhs = x^T chunk [c,tok].
            ps = psum.tile([128, 512], f32, tag=tag, name=f"qk_{tag}_{mb}_{nq}")
            for cc in range(CC):
                nc.tensor.matmul(
                    ps[:],
                    w_sb[:, cc, mb * 128:(mb + 1) * 128],
                    xt_sb[:, cc, nq * 512:(nq + 1) * 512],
                    start=(cc == 0), stop=(cc == CC - 1),
                )
            nc.vector.tensor_copy(dst[:, mb, nq * 512:(nq + 1) * 512], ps[:])

        for j in range(NQ):                      # K, all chunks
            for mb in range(2):
                qk_group(wk_sb, kt_sb, mb, j, f"stp{(2 * j + mb) % 2}")
        for mb in range(2):                      # Q chunk 0
            qk_group(wq_sb, qt_sb, mb, 0, f"stp{mb}")

        # ---- attention sweeps with woven projection -------------------------
        def s_exp(nq, kb):
            # Per head pair: S^T block via 2 row-tiled concurrent matmuls
            # (lhsT = K^T_h [d=64, k-block], rhs = Q^T_h), then one
            # P^T = exp(SCALE*S^T) over the pair's [128,1024] PSUM slot.
            qsl = slice(nq * 512, (nq + 1) * 512)
            pts_kb = []
            for pr in range(2):
                stp = psum.tile([128, 1024], f32, tag=f"stp{pr}",
                                name=f"stp{pr}_{nq}_{kb}")
                for hh in range(2):
                    h = 2 * pr + hh
                    po = 64 * (h % 2)
                    nc.tensor.matmul(
                        stp[:, hh * 512:(hh + 1) * 512],
                        kt_sb[po:po + 64, h // 2, kb * 128:(kb + 1) * 128],
                        qt_sb[po:po + 64, h // 2, qsl],
                        start=True, stop=True,
                    )
                pt = pts.tile([128, 1024], bf16, tag=f"pt{pr}",
                              name=f"pt{pr}_{nq}_{kb}")
                nc.scalar.activation(pt[:], stp[:], EXP, scale=SCALE)
                pts_kb.append(pt)
            return pts_kb

        def pv(accs, pts_kb, kb):
            for h in range(HPG):
                nc.tensor.matmul(
                    accs[h][:], v_sb[:, kb, h, :],
                    pts_kb[h // 2][:, (h % 2) * 512:(h % 2 + 1) * 512],
                    start=(kb == 0), stop=(kb == KB - 1),
                )

        def proj_mms(tb, nb, tag):
            ps = psum.tile([128, 512], f32, tag=tag, name=f"pjps{tb}_{nb}")
            for dc in range(2):
                nc.tensor.matmul(
                    ps[:],
                    ot_sb[:, dc, tb * 128:(tb + 1) * 128],
                    wp_sb[:, dc, nb * 512:(nb + 1) * 512],
                    start=(dc == 0), stop=(dc == 1),
                )
            return ps

        def norm_h(nq, accs, h, tail=False):
            qsl = slice(nq * 512, (nq + 1) * 512)
            po = 64 * (h % 2)
            den = small.tile([64, 512], f32, tag="den", name=f"den{h}_{nq}")
            if tail:
                nc.scalar.copy(den[:], accs[h][64:128, :])
            else:
                nc.vector.tensor_copy(den[:], accs[h][64:128, :])
            rec = small.tile([64, 512], f32, tag="rec", name=f"rec{h}_{nq}")
            nc.vector.reciprocal_approx_fast(rec[:], den[:])
            nc.vector.tensor_mul(
                ot_sb[po:po + 64, h // 2, qsl], accs[h][0:64, :], rec[:])

        def norm(nq, accs, tail=False):
            # ot = O^T * (1/l). The custom-DVE fast reciprocal (~51 ULP) is
            # broken for PSUM-src at base_partition 64 and for partition-
            # shifted outputs, so stage the denominator into SBUF with a plain
            # copy first (on ACT at the tail where it is idle, on DVE at the
            # boundaries where ACT is the bottleneck).
            for h in range(HPG):
                norm_h(nq, accs, h, tail)

        previews = []           # S/exp of the next sweep's first kbs, pre-issued
        for nq in range(NQ):
            accs = [psum.tile([128, 512], f32, tag=f"acc{h}", name=f"acc{h}_{nq}")
                    for h in range(HPG)]
            ob = None
            for kb in range(KB):
                pts_kb = previews.pop(0) if previews else s_exp(nq, kb)
                pv(accs, pts_kb, kb)
                # woven work (runs in the PE slack while ACT chews the exps):
                if nq == 0 and kb in (6, 9):
                    # Q chunk 1 into the stp rotation (pays its duration once).
                    qk_group(wq_sb, qt_sb, (0 if kb == 6 else 1), 1,
                             f"stp{0 if kb == 6 else 1}")
                if nq >= 1 and 2 <= kb <= 9:
                    # projection of the previous q-chunk: one psum-group per
                    # iteration, one output DMA per token-block on the idle
                    # GpSimd queue. (Keeping the slot alternation stp0/stp1 —
                    # pinning to stp1 removes the stalls but the denser PE
                    # stream then trips the P0 downclock: net 1.2x loss.)
                    tb = 4 * (nq - 1) + (kb - 2) // 2
                    nb = (kb - 2) % 2
                    if nb == 0:
                        ob = outst.tile([128, 1024], bf16, tag="ob",
                                        name=f"ob{tb}")
                    ps = proj_mms(tb, nb, f"stp{nb}")
                    nc.vector.tensor_copy(
                        ob[:, nb * 512:(nb + 1) * 512], ps[:])
                    if nb == 1:
                        nc.gpsimd.dma_start(
                            out[tb * 128:(tb + 1) * 128, :], ob[:])
            # ---- sweep boundary ---------------------------------------------
            # Pre-issue the next sweep's first two kbs of S/exp (~6 exps of
            # ACT buffer) so the exp stream never waits behind the norm ->
            # Q-weave -> PE chain below.
            if nq + 1 < NQ:
                previews = [s_exp(nq + 1, 0)]
            if nq < NQ - 1:
                norm(nq, accs)   # nq==3's norm happens dc-split in the tail
            # Q chunks 2/3 ride the boundary in the just-freed acc banks (the
            # next sweep's PVs have PE slack to absorb the delay).
            if nq + 2 <= NQ - 1:
                for mb in range(2):
                    qk_group(wq_sb, qt_sb, mb, nq + 2, f"acc{mb}")
        # ---- tail: norm of q-chunk 3, then its projection -------------------
        norm(NQ - 1, accs, tail=True)
        for i, tb in enumerate(range(4 * (NQ - 1), 4 * NQ)):
            ob = outst.tile([128, 1024], bf16, tag="ob", name=f"ob{tb}")
            for nb in range(2):
                ps = proj_mms(tb, nb, f"acc{(2 * i + nb) % 4}")
                nc.vector.tensor_copy(ob[:, nb * 512:(nb + 1) * 512], ps[:])
            eng = nc.gpsimd if i % 2 == 0 else nc.sync
            eng.dma_start(out[tb * 128:(tb + 1) * 128, :], ob[:])

    nc.compile()
    return nc


def get_nc():
    if "nc" not in _CACHE:
        _CACHE["nc"] = _build_nc()
    return _CACHE["nc"]


def to_bf16(a):
    import ml_dtypes
    return np.ascontiguousarray(np.asarray(a, dtype=np.float32)).astype(ml_dtypes.bfloat16)


def make_in_maps(x, w_qkv, w_proj, b_proj):
    x = np.ascontiguousarray(np.asarray(x, dtype=np.float32))
    w_qkv = np.asarray(w_qkv, dtype=np.float32)
    w_proj = np.asarray(w_proj, dtype=np.float32)
    b_proj = np.asarray(b_proj, dtype=np.float32)

    wr = w_qkv.reshape(DIM, 3, NUM_HEADS, HEAD_DIM)
    xts = [to_bf16(x[b].T) for b in range(B)]

    in_maps = []
    for core in range(NCORES):
        b, g = divmod(core, HPG)
        h0, h1 = HPG * g, HPG * (g + 1)
        in_maps.append({
            "xt": xts[b],
            "wq": to_bf16(wr[:, 0, h0:h1, :].reshape(DIM, GD)),
            "wk": to_bf16(wr[:, 1, h0:h1, :].reshape(DIM, GD)),
            "wv": to_bf16(wr[:, 2, h0:h1, :].reshape(DIM, GD)),
            "wp": to_bf16(w_proj[g * GD:(g + 1) * GD, :]),
        })
    return in_maps


def gather_out(results, b_proj):
    b_proj = np.asarray(b_proj, dtype=np.float32)
    parts = [np.asarray(r["out"], dtype=np.float32) for r in results]
    return np.stack(
        [sum(parts[b * HPG:(b + 1) * HPG][1:], parts[b * HPG]) + b_proj
         for b in range(B)],
        axis=0,
    ).astype(np.float32)


def kernel(x, w_qkv, w_proj, b_proj):
    from concourse import bass_utils

    nc = get_nc()
    in_maps = make_in_maps(x, w_qkv, w_proj, b_proj)
    res = bass_utils.run_bass_kernel_spmd(nc, in_maps, core_ids=list(range(NCORES)))
    return gather_out(res.results, b_proj)
